# revision 7
# baseline (speedup 1.0000x reference)
"""Trainium2 Bass kernel for 3x3 same-padding Conv2d + bias (NCHW).

Problem: x[16,32,256,256] (*) weight[32,32,3,3] + bias[32] -> out[16,32,256,256]

Strategy (data-parallel over batch, 2 images per NeuronCore on 8 cores):
  - Host pre-shuffles x into the SBUF "slot" layout x_shuf[b][(g,ci)][s][258]:
    image row h lives in row-group g=(h+1)%4 at slot s=(h+1)//4; each slot is
    258 wide (zero pad col on each side) so the 3 horizontal conv taps are
    plain free-dim shifts.  Device input DMAs are fully contiguous.
  - Output computed in "quads" of 4 consecutive rows: PSUM tile [128, 256]
    with partitions = (r, co).  Quad u accumulates 6 matmuls (K=128, M=128,
    N=256): 3 horizontal taps kw for the slot-u window (rows 4u-1..4u+2) and
    3 for the slot-(u+1) window (rows 4u+3..4u+4; other weight rows zero).
  - Weight matrices (6 x [128,128], zero-padded per (g, r_out, kh) validity)
    are precomputed on the host from `weight`.
  - Matmuls run as float32r (full-rate fp32, reduced-precision multiplies).
  - PSUM -> SBUF staging copies alternate VectorE/ScalarE; the device writes
    out_shuf[b][(r,co)][q][w] (contiguous per partition) and the host
    unshuffles to NCHW and adds bias (exact for any bias).
"""
import sys

if "/opt/trn_rl_repo" not in sys.path:
    sys.path.insert(0, "/opt/trn_rl_repo")

import numpy as np

B, C, H, W = 16, 32, 256, 256
N_CORES = 8
PER = B // N_CORES          # batches per core
HW = H * W
NSLOT = H // 4 + 1          # 65 row slots
SLOTW = W + 2               # 258 padded columns per slot
NQ = H // 4                 # 64 quads per image
CHUNK = 4                   # quads per staging buffer / out DMA
QSPLIT = 8                  # quads per input sub-tile (tile = QSPLIT+1 slots)

_cache = {}


def _get_nc():
    if "nc" in _cache:
        return _cache["nc"]
    import concourse.mybir as mybir
    import concourse.tile as tile
    import concourse.bass as bass
    from concourse import bacc

    DT = mybir.dt.float32r
    F32 = mybir.dt.float32

    nc = bacc.Bacc("TRN2", target_bir_lowering=False, debug=False,
                   num_devices=N_CORES)
    x_shuf = nc.dram_tensor("x_shuf", [PER, 128, NSLOT * SLOTW], DT,
                            kind="ExternalInput")
    w_taps = nc.dram_tensor("w_taps", [6, 128, 128], DT, kind="ExternalInput")
    out_shuf = nc.dram_tensor("out_shuf", [PER, 128, NQ * W], F32,
                              kind="ExternalOutput")

    NSUB = NQ // QSPLIT     # input sub-tiles per batch
    with tile.TileContext(nc) as tc:
        with (
            tc.tile_pool(name="xin", bufs=PER * NSUB) as xpool,
            tc.tile_pool(name="wts", bufs=1) as wpool,
            tc.tile_pool(name="stage", bufs=3) as spool,
            tc.tile_pool(name="psum", bufs=8, space="PSUM") as ppool,
        ):
            w_t = wpool.tile([128, 6, 128], DT)
            nc.sync.dma_start(out=w_t[:],
                              in_=w_taps.ap().rearrange("t k m -> k t m"))

            # load both batches up front as contiguous sub-tiles of
            # QSPLIT+1 slots (1-slot overlap) so compute starts after the
            # first ~2 MB instead of the full 17 MB
            xts = []
            for b in range(PER):
                for j in range(NSUB):
                    xt = xpool.tile([128, QSPLIT + 1, SLOTW], DT)
                    lo = j * QSPLIT * SLOTW
                    hi = lo + (QSPLIT + 1) * SLOTW
                    nc.sync.dma_start(
                        out=xt[:],
                        in_=x_shuf.ap()[b, :, lo:hi]
                        .rearrange("p (s w) -> p s w", w=SLOTW))
                    xts.append(xt)

            for b in range(PER):
                for k in range(NQ // CHUNK):
                    st = spool.tile([128, CHUNK, W], F32)
                    for ql in range(CHUNK):
                        u = k * CHUNK + ql
                        xt = xts[b * NSUB + u // QSPLIT]
                        lu = u % QSPLIT
                        ps = ppool.tile([128, W], F32)
                        for kw in range(3):
                            nc.tensor.matmul(ps[:], w_t[:, kw * 2, :],
                                             xt[:, lu, kw:kw + W],
                                             start=(kw == 0), stop=False)
                            nc.tensor.matmul(ps[:], w_t[:, kw * 2 + 1, :],
                                             xt[:, lu + 1, kw:kw + W],
                                             start=False, stop=(kw == 2))
                        nc.vector.tensor_copy(st[:, ql, :], ps[:])
                    # contiguous per-partition store of CHUNK quads; issued
                    # on the ACT HWDGE ring so stores never queue ahead of
                    # input loads (which use the SP ring)
                    dst = bass.AP(out_shuf, b * 128 * NQ * W + k * CHUNK * W,
                                  [[NQ * W, 128], [1, CHUNK * W]])
                    nc.scalar.dma_start(
                        out=dst,
                        in_=st[:].rearrange("p q w -> p (q w)"))

    nc.compile()
    _cache["nc"] = nc
    return nc


def _make_w_taps(weight):
    """Zero-padded stationary matrices w_taps[kw*2+part][(g,ci), (r,co)]."""
    w_taps = np.zeros((6, 128, 128), dtype=np.float32)
    for kw in range(3):
        for g in range(4):
            for r in range(4):
                kh0 = g - r              # window W_u (input row 4u+g-1)
                if 0 <= kh0 <= 2:
                    w_taps[kw * 2, g * 32:(g + 1) * 32, r * 32:(r + 1) * 32] = \
                        weight[:, :, kh0, kw].T
                kh1 = g - r + 4          # window W_{u+1} (input row 4u+g+3)
                if 0 <= kh1 <= 2:
                    w_taps[kw * 2 + 1, g * 32:(g + 1) * 32, r * 32:(r + 1) * 32] = \
                        weight[:, :, kh1, kw].T
    return w_taps


def _shuffle_x(x):
    """x[B,C,H,W] -> x_shuf[B,128,NSLOT,SLOTW]: row h -> (group (h+1)%4,
    slot (h+1)//4), cols 1..W, zero pads elsewhere."""
    xs = np.zeros((B, 128, NSLOT, SLOTW), dtype=np.float32)
    # group g, slot s holds row 4s+g-1
    xs[:, 0:32, 1:NSLOT, 1:W + 1] = x[:, :, 3::4, :].transpose(0, 1, 2, 3)
    xs[:, 32:64, 0:NSLOT - 1, 1:W + 1] = x[:, :, 0::4, :]
    xs[:, 64:96, 0:NSLOT - 1, 1:W + 1] = x[:, :, 1::4, :]
    xs[:, 96:128, 0:NSLOT - 1, 1:W + 1] = x[:, :, 2::4, :]
    return xs.reshape(B, 128, NSLOT * SLOTW)


def _unshuffle_out(chunks):
    """chunks: list of PER-core arrays [PER,128,NQ*W] -> out[B,C,H,W]."""
    o = np.concatenate(chunks, axis=0)              # [B, 128, NQ*W]
    o = o.reshape(B, 4, C, NQ, W)                   # [(r c), q, w]
    o = o.transpose(0, 2, 3, 1, 4)                  # [B, C, q, r, w]
    return np.ascontiguousarray(o.reshape(B, C, H, W))


def kernel(x, weight, bias):
    from concourse.bass_utils import run_bass_kernel_spmd

    x = np.asarray(x, dtype=np.float32)
    weight = np.asarray(weight, dtype=np.float32)
    bias = np.asarray(bias, dtype=np.float32)

    nc = _get_nc()
    w_taps = _make_w_taps(weight)
    x_shuf = _shuffle_x(x)
    in_maps = [{"x_shuf": x_shuf[c * PER:(c + 1) * PER], "w_taps": w_taps}
               for c in range(N_CORES)]
    res = run_bass_kernel_spmd(nc, in_maps, list(range(N_CORES)))
    out = _unshuffle_out([res.results[c]["out_shuf"] for c in range(N_CORES)])
    out += bias.reshape(1, C, 1, 1)
    return out


# revision 12
# speedup vs baseline: 1.2051x; 1.2051x over previous
"""Trainium2 Bass kernel for 3x3 same-padding Conv2d + bias (NCHW).

Problem: x[16,32,256,256] (*) weight[32,32,3,3] + bias[32] -> out[16,32,256,256]

Strategy (data-parallel over batch, 2 images per NeuronCore on 8 cores):
  - Host pre-shuffles x into the SBUF "slot" layout x_shuf[b][(g,ci)][s][258]:
    image row h lives in row-group g=(h+1)%4 at slot s=(h+1)//4; each slot is
    258 wide (zero pad col on each side) so the 3 horizontal conv taps are
    plain free-dim shifts.  Device input DMAs are fully contiguous.
  - Output computed in "quads" of 4 consecutive rows: PSUM tile [128, 256]
    with partitions = (r, co).  Quad u accumulates 6 matmuls (K=128, M=128,
    N=256): 3 horizontal taps kw for the slot-u window (rows 4u-1..4u+2) and
    3 for the slot-(u+1) window (rows 4u+3..4u+4; other weight rows zero).
  - Weight matrices (6 x [128,128], zero-padded per (g, r_out, kh) validity)
    are precomputed on the host from `weight`.
  - Matmuls run as float32r (full-rate fp32, reduced-precision multiplies).
  - PSUM -> SBUF staging copies alternate VectorE/ScalarE; the device writes
    out_shuf[b][(r,co)][q][w] (contiguous per partition) and the host
    unshuffles to NCHW and adds bias (exact for any bias).
"""
import sys

if "/opt/trn_rl_repo" not in sys.path:
    sys.path.insert(0, "/opt/trn_rl_repo")

import numpy as np

B, C, H, W = 16, 32, 256, 256
N_CORES = 8
PER = B // N_CORES          # batches per core
HW = H * W
NSLOT = H // 4 + 1          # 65 row slots
SLOTW = W + 2               # 258 padded columns per slot
NQ = H // 4                 # 64 quads per image
CHUNK = 4                   # quads per staging buffer / out DMA
QSPLIT = 16                 # quads per input sub-tile (tile = QSPLIT+1 slots)

DT_KEY = "fp32r"            # "fp32r" (exact-ish) or "bf16" (half input DMA)

_cache = {}


def _get_nc(dt_key=None):
    dt_key = dt_key or DT_KEY
    if dt_key in _cache:
        return _cache[dt_key]
    import concourse.mybir as mybir
    import concourse.tile as tile
    import concourse.bass as bass
    from concourse import bacc

    DT = mybir.dt.float32r if dt_key == "fp32r" else mybir.dt.bfloat16
    F32 = mybir.dt.float32

    nc = bacc.Bacc("TRN2", target_bir_lowering=False, debug=False,
                   num_devices=N_CORES)
    x_shuf = nc.dram_tensor("x_shuf", [PER, 128, NSLOT * SLOTW], DT,
                            kind="ExternalInput")
    w_taps = nc.dram_tensor("w_taps", [6, 128, 128], DT, kind="ExternalInput")
    out_shuf = nc.dram_tensor("out_shuf", [PER, 128, NQ * W], F32,
                              kind="ExternalOutput")

    NSUB = NQ // QSPLIT     # input sub-tiles per batch
    with tile.TileContext(nc) as tc:
        with (
            tc.tile_pool(name="xin", bufs=PER * NSUB) as xpool,
            tc.tile_pool(name="wts", bufs=1) as wpool,
            tc.tile_pool(name="stage", bufs=3) as spool,
            tc.tile_pool(name="psum", bufs=8, space="PSUM") as ppool,
        ):
            w_t = wpool.tile([128, 6, 128], DT)
            nc.sync.dma_start(out=w_t[:],
                              in_=w_taps.ap().rearrange("t k m -> k t m"))

            # load both batches up front as contiguous sub-tiles of
            # QSPLIT+1 slots (1-slot overlap) so compute starts after the
            # first ~2 MB instead of the full 17 MB
            xts = []
            for b in range(PER):
                for j in range(NSUB):
                    xt = xpool.tile([128, QSPLIT + 1, SLOTW], DT)
                    lo = j * QSPLIT * SLOTW
                    hi = lo + (QSPLIT + 1) * SLOTW
                    nc.sync.dma_start(
                        out=xt[:],
                        in_=x_shuf.ap()[b, :, lo:hi]
                        .rearrange("p (s w) -> p s w", w=SLOTW))
                    xts.append(xt)

            for b in range(PER):
                for k in range(NQ // CHUNK):
                    st = spool.tile([128, CHUNK, W], F32)
                    for ql in range(CHUNK):
                        u = k * CHUNK + ql
                        xt = xts[b * NSUB + u // QSPLIT]
                        lu = u % QSPLIT
                        ps = ppool.tile([128, W], F32)
                        for kw in range(3):
                            nc.tensor.matmul(ps[:], w_t[:, kw * 2, :],
                                             xt[:, lu, kw:kw + W],
                                             start=(kw == 0), stop=False)
                            nc.tensor.matmul(ps[:], w_t[:, kw * 2 + 1, :],
                                             xt[:, lu + 1, kw:kw + W],
                                             start=False, stop=(kw == 2))
                        nc.vector.tensor_copy(st[:, ql, :], ps[:])
                    # contiguous per-partition store of CHUNK quads; issued
                    # on the ACT HWDGE ring so stores never queue ahead of
                    # input loads (which use the SP ring)
                    dst = bass.AP(out_shuf, b * 128 * NQ * W + k * CHUNK * W,
                                  [[NQ * W, 128], [1, CHUNK * W]])
                    nc.scalar.dma_start(
                        out=dst,
                        in_=st[:].rearrange("p q w -> p (q w)"))

    nc.compile()
    _cache[dt_key] = nc
    return nc


def _make_w_taps(weight):
    """Zero-padded stationary matrices w_taps[kw*2+part][(g,ci), (r,co)]."""
    w_taps = np.zeros((6, 128, 128), dtype=np.float32)
    for kw in range(3):
        for g in range(4):
            for r in range(4):
                kh0 = g - r              # window W_u (input row 4u+g-1)
                if 0 <= kh0 <= 2:
                    w_taps[kw * 2, g * 32:(g + 1) * 32, r * 32:(r + 1) * 32] = \
                        weight[:, :, kh0, kw].T
                kh1 = g - r + 4          # window W_{u+1} (input row 4u+g+3)
                if 0 <= kh1 <= 2:
                    w_taps[kw * 2 + 1, g * 32:(g + 1) * 32, r * 32:(r + 1) * 32] = \
                        weight[:, :, kh1, kw].T
    return w_taps


def _shuffle_x(x, np_dt=np.float32):
    """x[B,C,H,W] -> x_shuf[B,128,NSLOT,SLOTW]: row h -> (group (h+1)%4,
    slot (h+1)//4), cols 1..W, zero pads elsewhere."""
    xs = np.zeros((B, 128, NSLOT, SLOTW), dtype=np_dt)
    # group g, slot s holds row 4s+g-1
    xs[:, 0:32, 1:NSLOT, 1:W + 1] = x[:, :, 3::4, :].astype(np_dt)
    xs[:, 32:64, 0:NSLOT - 1, 1:W + 1] = x[:, :, 0::4, :].astype(np_dt)
    xs[:, 64:96, 0:NSLOT - 1, 1:W + 1] = x[:, :, 1::4, :].astype(np_dt)
    xs[:, 96:128, 0:NSLOT - 1, 1:W + 1] = x[:, :, 2::4, :].astype(np_dt)
    return xs.reshape(B, 128, NSLOT * SLOTW)


def _unshuffle_out(chunks):
    """chunks: list of PER-core arrays [PER,128,NQ*W] -> out[B,C,H,W]."""
    o = np.concatenate(chunks, axis=0)              # [B, 128, NQ*W]
    o = o.reshape(B, 4, C, NQ, W)                   # [(r c), q, w]
    o = o.transpose(0, 2, 3, 1, 4)                  # [B, C, q, r, w]
    return np.ascontiguousarray(o.reshape(B, C, H, W))


def _np_dt(dt_key):
    if dt_key == "fp32r":
        return np.float32
    import ml_dtypes
    return ml_dtypes.bfloat16


def make_in_maps(x, weight, dt_key=None):
    dt_key = dt_key or DT_KEY
    np_dt = _np_dt(dt_key)
    w_taps = _make_w_taps(np.asarray(weight, dtype=np.float32)).astype(np_dt)
    x_shuf = _shuffle_x(np.asarray(x, dtype=np.float32), np_dt)
    return [{"x_shuf": x_shuf[c * PER:(c + 1) * PER], "w_taps": w_taps}
            for c in range(N_CORES)]


def kernel(x, weight, bias):
    from concourse.bass_utils import run_bass_kernel_spmd

    bias = np.asarray(bias, dtype=np.float32)
    nc = _get_nc()
    in_maps = make_in_maps(x, weight)
    res = run_bass_kernel_spmd(nc, in_maps, list(range(N_CORES)))
    out = _unshuffle_out([res.results[c]["out_shuf"] for c in range(N_CORES)])
    out += bias.reshape(1, C, 1, 1)
    return out
